# revision 1
# baseline (speedup 1.0000x reference)
"""Trainium2 Bass/Tile kernel: batched dot-product attention with length masking.

Problem: queries/keys/values [32, 1024, 128] f32, valid_length [32] int64.
  out = softmax(mask(Q K^T / sqrt(128))) @ V

Strategy:
  - Data-parallel: 32 batches sharded 4-per-core across 8 NeuronCores (SPMD,
    identical program, per-core input maps).
  - Host prep per batch (layout only; every tensor is a single fully
    contiguous DMA so descriptors aggregate into large packets):
      qT      [128=D, 1024] f32->fp16  (contraction dim on partitions)
      k{b}    [128=D, KC]   fp16       (K^T trimmed to the live k-blocks)
      v{b}    [128, KB*128] fp16       (V partition-major per k-block)
      fb{b}   [128, KB]     f32        exp-bias: 0 for valid k, -1e4 masked
  - Device per batch (matmul passes stream 512-row moving operands so the
    PE keeps its stationary loaded across 1024 rows):
      S^T[k, q] = (K^T_kb).T @ Q^T           PE
      P^T_kb    = exp(S^T*scale + fb[:,kb])  ScalarE PSUM->SBUF fp16.
                  The per-partition bias is -1e4 on masked k rows, so exp
                  underflows to exactly 0 there: masking costs nothing and
                  no separate mask matmul or V-zeroing is needed.
      pacc      = sum_kb P^T_kb              DVE adds (cheap, off PE)
      den[1,q]  = ones.T @ pacc              PE, only 2x512 rows per batch
                                             (vs KB*2x512 for a full
                                             mask-stationary den pass)
      O^T[v,q]  = sum_kb V_kb @ P^T_kb       PE, V stationary
    The last batch skips pacc and accumulates den over the P tiles directly
    on the PE (KB is smallest there after the sort) so the tail has no
    DVE dependency. Host does out = O^T.T / den in f32.
    No rowmax subtraction needed: scores ~ N(0,1), |S*scale| <~ 6.
  - DMA issues avoid GpSimd entirely: its DGE ring is software-managed and
    costs ~3us in the end-of-kernel drain (sync/scalar/vector rings are HW).
    k0 goes on scalar + q0 on sync so both batch-0 S operands issue in
    parallel at t=0; v's on vector; everything else on sync, with batch b+1
    loads emitted before den_pv(b-1) so output DMAs never delay loads.
  - A dummy 1-column exp at kernel start pulls the ~1.3us ACT_TABLE_LOAD
    into the initial DMA shadow (the compiler inserts it before the first
    Exp on the scalar engine).
  - Length specialization: batches sorted by valid_length desc, assigned
    round-robin so slot j is similar across cores; program compiled per
    kb_counts skips fully-masked k-blocks.
"""

import os

import numpy as np
import ml_dtypes

import concourse.tile as tile
from concourse import bacc, mybir
from concourse.bass_utils import run_bass_kernel_spmd

B, Q, K, D = 32, 1024, 1024, 128
N_CORES = 8
BPC = B // N_CORES  # batches per core
KB_MAX = K // 128
QH = 512
SCALE = float(1.0 / np.sqrt(D))
MASK_BIAS = -10000.0  # exp(s*scale + MASK_BIAS) underflows to exactly 0

S_DTYPE = os.environ.get("ATTN_S_DTYPE", "fp16")  # fp16 | bf16 | f32r | f32
NO_SPECIALIZE = os.environ.get("ATTN_NO_SPECIALIZE", "0") == "1"
N_WARM = int(os.environ.get("ATTN_WARM", "8"))

LAST_RESULTS = None
_NC_CACHE: dict = {}


def _dtypes(sdt):
    """(qk_dt for Q/K/S-matmul, ldt for P/V/ones)."""
    f32 = mybir.dt.float32
    qk = {"fp16": mybir.dt.float16, "bf16": mybir.dt.bfloat16,
          "f32r": mybir.dt.float32r, "f32": f32}[sdt]
    ldt = mybir.dt.float16 if sdt == "fp16" else mybir.dt.bfloat16
    return qk, ldt


def _body(tc, qT, kts, vts, fbs, outT, den, kb_counts, sdt):
    nc = tc.nc
    f32 = mybir.dt.float32
    AF = mybir.ActivationFunctionType
    qk_dt, ldt = _dtypes(sdt)

    with (
        tc.tile_pool(name="qk", bufs=3) as qk_pool,
        tc.tile_pool(name="v", bufs=3) as v_pool,
        tc.tile_pool(name="p", bufs=2) as p_pool,
        tc.tile_pool(name="pa", bufs=2) as pa_pool,
        tc.tile_pool(name="fb", bufs=3) as fb_pool,
        tc.tile_pool(name="eps", bufs=2) as e_pool,
        tc.tile_pool(name="const", bufs=1) as c_pool,
        tc.tile_pool(name="spsum", bufs=2, space="PSUM") as s_pool,
        tc.tile_pool(name="opsum", bufs=1, space="PSUM") as o_pool,
        tc.tile_pool(name="dpsum", bufs=1, space="PSUM") as d_pool,
    ):
        KBM = max(kb_counts)

        def load_batch(b):
            # per-tag tile shapes are constant (max KB) so the pool slot
            # size doesn't depend on allocation order; DMAs/compute slice
            KB = kb_counts[b]
            KC = KB * 128
            q_sb = qk_pool.tile([128, Q], qk_dt, tag="q", name=f"q_sb{b}")
            k_sb = qk_pool.tile([128, KBM * 128], qk_dt, tag="k",
                                name=f"k_sb{b}")
            v_sb = v_pool.tile([128, KBM * 128], ldt, tag="v",
                               name=f"v_sb{b}")
            fb_sb = fb_pool.tile([128, KBM], f32, tag="fb",
                                 name=f"fb_sb{b}")
            # two HW DGE rings (only gpsimd/SP/ACT can issue DMAs; gpsimd's
            # software ring costs ~3us in the final drain, so it issues
            # nothing). Slots are ordered smallest-KB first, so batch 0's k
            # is tiny: k0 rides sync while the full q0 rides scalar, and
            # both batch-0 S operands land ~3.5us (the fixed DGE latency)
            # after kernel entry. k1-k3 keep the scalar ring busy before its
            # exp stream starts.
            if b == 0:
                nc.sync.dma_start(out=k_sb[:, 0:KC], in_=kts[b][:])
                nc.scalar.dma_start(out=q_sb[:], in_=qT[b])
                nc.sync.dma_start(out=fb_sb[:, 0:KB], in_=fbs[b][:])
                nc.scalar.dma_start(out=v_sb[:, 0:KC], in_=vts[b][:])
                return q_sb, k_sb, v_sb, fb_sb
            elif b == 1:
                # k1 is needed early; the scalar ring is still busy with q0.
                # Completion is ~issue-end + 3.5us fixed DGE latency, so
                # queue position directly sets arrival time.
                nc.sync.dma_start(out=k_sb[:, 0:KC], in_=kts[b][:])
                nc.sync.dma_start(out=q_sb[:], in_=qT[b])
            else:
                nc.scalar.dma_start(out=k_sb[:, 0:KC], in_=kts[b][:])
                nc.sync.dma_start(out=q_sb[:], in_=qT[b])
            nc.sync.dma_start(out=fb_sb[:, 0:KB], in_=fbs[b][:])
            nc.sync.dma_start(out=v_sb[:, 0:KC], in_=vts[b][:])
            return q_sb, k_sb, v_sb, fb_sb

        def s_exp_one(b, kb, q_sb, k_sb, fb_sb, p_all):
            s_ps = s_pool.tile([128, Q], f32, tag="s", name=f"s_ps{b}_{kb}")
            lhsT = k_sb[:, kb * 128 : (kb + 1) * 128]
            for qh in range(Q // QH):
                nc.tensor.matmul(
                    s_ps[:, qh * QH : (qh + 1) * QH],
                    lhsT,
                    q_sb[:, qh * QH : (qh + 1) * QH],
                    start=True,
                    stop=True,
                )
            p_kb = p_all[:, kb * Q : (kb + 1) * Q]
            nc.scalar.activation(p_kb, s_ps[:], AF.Exp, scale=SCALE,
                                 bias=fb_sb[:, kb : kb + 1])

        # s_exp is split in a head (first two k-blocks, no DVE work) and a
        # tail: the head of batch b+1 is emitted before den_pv(b), so the
        # ScalarE exp stream never starves at a batch boundary (exp(b+1,0)
        # only needs S(b+1,0), which the PE runs right after S(b)'s tail),
        # while den_pv(b)'s PE/DVE work still fills the PSUM-recycle waits
        # and lands before batch b+1's DVE adds.
        def s_exp_head(b, q_sb, k_sb, fb_sb):
            KB = kb_counts[b]
            p_all = p_pool.tile([128, KBM * Q], ldt, tag="p", name=f"p{b}")
            for kb in range(min(2, KB)):
                s_exp_one(b, kb, q_sb, k_sb, fb_sb, p_all)
            return p_all

        def s_exp_tail(b, q_sb, k_sb, fb_sb, p_all, start_kb):
            KB = kb_counts[b]
            if KB == 1:
                return None
            pacc = pa_pool.tile([128, Q], ldt, tag="pa", name=f"pa{b}")
            nc.vector.tensor_add(pacc[:], p_all[:, 0:Q], p_all[:, Q : 2 * Q])
            for kb in range(2, KB):
                if kb >= start_kb:
                    s_exp_one(b, kb, q_sb, k_sb, fb_sb, p_all)
                # accumulate P tiles for the denominator as soon as each exp
                # lands; the DVE chain trails the ScalarE stream
                nc.vector.tensor_add(
                    pacc[:], pacc[:], p_all[:, kb * Q : (kb + 1) * Q])
            return pacc

        def den_pv_stage(b, p_all, v_sb, pacc, nxt=None):
            KB = kb_counts[b]
            last = b == BPC - 1
            # O^T[v, q] accumulated over k-blocks, V stationary (kb-outer)
            o_ps = [o_pool.tile([128, QH], f32, tag=f"o{qh}", name=f"o_ps{b}_{qh}")
                    for qh in range(Q // QH)]
            for kb in range(KB):
                for qh in range(Q // QH):
                    nc.tensor.matmul(
                        o_ps[qh][:],
                        v_sb[:, kb * 128 : (kb + 1) * 128],
                        p_all[:, kb * Q + qh * QH : kb * Q + (qh + 1) * QH],
                        start=(kb == 0),
                        stop=(kb == KB - 1),
                    )
            # next batch's 3rd S tile + exp go here: after PV (so nothing
            # parks the in-order PE queue on its PSUM-recycle wait) but
            # before den, keeping the ScalarE stream dense across the
            # batch boundary
            if nxt is not None:
                nb, nq, nk, nfb, np_all = nxt
                if kb_counts[nb] > 2:
                    s_exp_one(nb, 2, nq, nk, nfb, np_all)
            # denominator: one moving pass over the accumulated P, ones
            # stationary (2x512 rows vs KB*2x512 for a mask-matmul pass)
            d_ps = d_pool.tile([1, Q], f32, tag="d", name=f"d_ps{b}")
            dsrc = pacc if pacc is not None else p_all
            for qh in range(Q // QH):
                nc.tensor.matmul(
                    d_ps[:, qh * QH : (qh + 1) * QH],
                    ones_sb[:, 0:1],
                    dsrc[:, qh * QH : (qh + 1) * QH],
                    start=True,
                    stop=True,
                )
            # PSUM can't DMA directly and only ACT/DVE can read PSUM.
            den_sb = e_pool.tile([1, Q], f32, tag="densb", name=f"den_sb{b}")
            o_all = e_pool.tile([128, Q], ldt, tag="oall", name=f"o_all{b}")
            if last:
                # tail ordering: the big O^T halves evac and fly first, the
                # casts split across DVE and Scalar (its exp stream is done),
                # then the small den halves follow on two DGE rings
                nc.vector.tensor_copy(o_all[:, 0:QH], o_ps[0][:])
                nc.sync.dma_start(out=outT[b][:, 0:QH], in_=o_all[:, 0:QH])
                nc.scalar.copy(o_all[:, QH:Q], o_ps[1][:])
                nc.scalar.dma_start(out=outT[b][:, QH:Q], in_=o_all[:, QH:Q])
                nc.vector.tensor_copy(den_sb[:, 0:QH], d_ps[:, 0:QH])
                nc.sync.dma_start(out=den[b][:, 0:QH], in_=den_sb[:, 0:QH])
                nc.scalar.copy(den_sb[:, QH:Q], d_ps[:, QH:Q])
                nc.scalar.dma_start(out=den[b][:, QH:Q], in_=den_sb[:, QH:Q])
            else:
                nc.vector.tensor_copy(den_sb[:], d_ps[:])
                nc.sync.dma_start(out=den[b], in_=den_sb[:])
                # evac with fp16 conversion on DVE: halves the output DMA
                # bytes; the host divides by den in f32 anyway. Single
                # fully-contiguous DMA -> large packets.
                for qh in range(Q // QH):
                    nc.vector.tensor_copy(
                        o_all[:, qh * QH : (qh + 1) * QH], o_ps[qh][:])
                nc.sync.dma_start(out=outT[b], in_=o_all[:])

        # batch-0 loads are emitted first so the k0 issue is the scalar
        # engine's first instruction (its exp work all comes later)
        loads = [load_batch(0)]

        # ones column for the denominator matmul
        ones_sb = c_pool.tile([128, 1], ldt, tag="ones", bufs=1)
        nc.gpsimd.memset(ones_sb[:], 1.0)
        # dummy 1-column exp: hoists the compiler-inserted ACT_TABLE_LOAD
        # (~1.3us) into the batch-0 DMA shadow
        scratch = c_pool.tile([128, 1], ldt, tag="scratch", bufs=1)
        nc.scalar.activation(scratch[:], ones_sb[:], AF.Exp, scale=1.0)

        # HAM pre-warm: dummy matmuls with no data deps run while the batch-0
        # loads are in flight, ramping the PE p-state (a cold PE runs its
        # first ~3us at reduced clock) and covering the DMA latency.
        warm_w = c_pool.tile([128, QH], qk_dt, tag="warmw", bufs=1)
        nc.gpsimd.memset(warm_w[:], 0.0)
        for w in range(N_WARM):
            warm_ps = s_pool.tile([128, QH], f32, tag="s", name=f"warm{w}")
            nc.tensor.matmul(warm_ps[:], warm_w[:, 0:128], warm_w[:],
                             start=True, stop=True)

        # Software pipeline (see s_exp_head comment): per iteration, emit
        # load(b+1), s_exp_tail(b), s_exp_head(b+1), den_pv(b).
        p_alls = [s_exp_head(0, loads[0][0], loads[0][1], loads[0][3])]
        for b in range(BPC):
            if b + 1 < BPC:
                loads.append(load_batch(b + 1))
            q_sb, k_sb, v_sb, fb_sb = loads[b]
            # batch 0's kb=2 tile has no preceding den_pv to ride in
            pacc = s_exp_tail(b, q_sb, k_sb, fb_sb, p_alls[b],
                              start_kb=2 if b == 0 else 3)
            nxt = None
            if b + 1 < BPC:
                lq, lk, lv, lfb = loads[b + 1]
                p_alls.append(s_exp_head(b + 1, lq, lk, lfb))
                nxt = (b + 1, lq, lk, lfb, p_alls[b + 1])
            den_pv_stage(b, p_alls[b], v_sb, pacc, nxt)


def _build(kb_counts, sdt):
    key = (tuple(kb_counts), sdt)
    if key in _NC_CACHE:
        return _NC_CACHE[key]
    nc = bacc.Bacc("TRN2", target_bir_lowering=False, debug=False,
                   enable_asserts=False, enable_partition_id=False)
    f32 = mybir.dt.float32
    qk_dt, ldt = _dtypes(sdt)
    qT = nc.dram_tensor("qT", [BPC, D, Q], qk_dt, kind="ExternalInput").ap()
    kts, vts, fbs = [], [], []
    for b in range(BPC):
        KC = kb_counts[b] * 128
        kts.append(nc.dram_tensor(f"k{b}", [D, KC], qk_dt,
                                  kind="ExternalInput").ap())
        vts.append(nc.dram_tensor(f"v{b}", [128, KC], ldt,
                                  kind="ExternalInput").ap())
        fbs.append(nc.dram_tensor(f"fb{b}", [128, kb_counts[b]], f32,
                                  kind="ExternalInput").ap())
    outT = nc.dram_tensor("outT", [BPC, D, Q], ldt, kind="ExternalOutput").ap()
    den = nc.dram_tensor("den", [BPC, 1, Q], f32, kind="ExternalOutput").ap()
    with tile.TileContext(nc) as tc:
        _body(tc, qT, kts, vts, fbs, outT, den, kb_counts, sdt)
    nc.compile()
    _NC_CACHE[key] = nc
    return nc


def _prep(queries, keys, values, valid_length):
    """Returns (in_maps, assign, kb_counts). assign[j, c] = original batch index
    handled by core c slot j."""
    vl = np.asarray(valid_length).astype(np.int64).reshape(B)
    if NO_SPECIALIZE:
        assign = np.arange(B).reshape(N_CORES, BPC).T
        kb_counts = tuple([KB_MAX] * BPC)
    else:
        # sort desc so each slot groups similar lengths across cores, then
        # process slots smallest-KB first: batch 0's k is tiny (arrives
        # fast, exp stream starts early) and later big k's hide behind
        # earlier compute
        order = np.argsort(-vl, kind="stable")
        assign = order.reshape(BPC, N_CORES)[::-1]  # [slot, core]
        kb_counts = tuple(
            max(1, int(np.ceil(vl[assign[j]].max() / 128.0))) for j in range(BPC)
        )

    qk_np = {"fp16": np.float16, "bf16": ml_dtypes.bfloat16,
             "f32r": np.float32, "f32": np.float32}[S_DTYPE]
    ldt_np = np.float16 if S_DTYPE == "fp16" else ml_dtypes.bfloat16
    q = np.asarray(queries, dtype=np.float32)
    k = np.asarray(keys, dtype=np.float32)
    v = np.asarray(values, dtype=np.float32)
    pos = np.arange(K)

    in_maps = []
    for c in range(N_CORES):
        bidx = assign[:, c]
        qTc = np.ascontiguousarray(q[bidx].transpose(0, 2, 1)).astype(qk_np)
        m = {"qT": qTc}
        for j in range(BPC):
            bi = bidx[j]
            KB = kb_counts[j]
            KC = KB * 128
            m[f"k{j}"] = np.ascontiguousarray(
                k[bi, :KC].T).astype(qk_np)  # [D, KC]
            m[f"v{j}"] = np.ascontiguousarray(
                v[bi, :KC].reshape(KB, 128, D).transpose(1, 0, 2).reshape(
                    128, KC)).astype(ldt_np)
            fb = np.where(pos[:KC] < vl[bi], 0.0, MASK_BIAS).astype(np.float32)
            m[f"fb{j}"] = np.ascontiguousarray(
                fb.reshape(KB, 128).T)  # [128, KB]
        in_maps.append(m)
    return in_maps, assign, kb_counts


def kernel(queries, keys, values, valid_length):
    global LAST_RESULTS
    in_maps, assign, kb_counts = _prep(queries, keys, values, valid_length)
    nc = _build(kb_counts, S_DTYPE)
    res = run_bass_kernel_spmd(nc, in_maps, list(range(N_CORES)))
    LAST_RESULTS = res
    out = np.empty((B, Q, D), np.float32)
    for c in range(N_CORES):
        oT = np.asarray(res.results[c]["outT"]).astype(np.float32)  # [BPC,D,Q]
        den = np.asarray(res.results[c]["den"], dtype=np.float32)  # [BPC, 1, Q]
        o = (oT / den).transpose(0, 2, 1)
        for j in range(BPC):
            out[assign[j, c]] = o[j]
    return out



# revision 3
# speedup vs baseline: 1.0628x; 1.0628x over previous
"""Trainium2 Bass/Tile kernel: batched dot-product attention with length masking.

Problem: queries/keys/values [32, 1024, 128] f32, valid_length [32] int64.
  out = softmax(mask(Q K^T / sqrt(128))) @ V

Strategy (v2 — balanced k-block packing, host-side denominator):
  - Work unit = one 128-wide k-block of one batch. Total blocks
    N = sum_b ceil(vl_b/128) (=136 for the reference input). Attention is
    associative over k (partial numerator + partial sum-of-exp combine on
    host), so blocks of one batch can be split across cores at will.
  - SPMD program = G slots with fixed block counts sizes[g]
    (sum = ceil(N/8) = 17). A DP packer cuts batches into contiguous
    k-range pieces so every core's slot g holds exactly sizes[g] blocks
    (padding pieces are all-zero and harmless). Per-core work is 17 blocks
    vs 20 for the per-slot-max batch scheme.
  - Masking costs nothing on device: host zeroes masked K columns and V
    rows. Then S=0 and exp(0)=1 on masked columns; the numerator is clean
    (V rows are 0) and the denominator over-counts by exactly the masked
    column count, which the host subtracts. No bias tensors, uniform exps.
  - Device per piece (q [128,Q] fp16 = Q^T, k [128,s*128] fp16 = K^T,
    v [128, s*128] fp16 V partition-major per block):
      S^T[k,q] = K_blk.T @ Q^T          PE (512-row moving passes)
      P^T_blk  = exp(S^T*scale)         ScalarE PSUM->SBUF fp16
      pacc     = sum_blk P^T_blk        DVE adds (s-1 per piece)
      O^T[v,q] = sum_blk V_blk @ P^T    PE, V stationary, PSUM accumulate
    Outputs per piece: oT [128,Q] fp16 and pacc [128,Q] fp16 (P itself for
    1-block pieces). Host: den = pacc.sum(partitions) - n_masked;
    out = sum_pieces oT^T / den. No den matmul on the PE and no PSUM bank
    for it -> s_pool gets 3 PSUM buffers (deeper S pipelining).
  - ScalarE runs ONLY the exp stream (the critical path: 17 x ~1.04us);
    all steady-state DMA issues ride the sync ring. Head: k0 on sync,
    q0 halves on scalar+vector in parallel so the first exp fires ~4.5us
    after kernel entry. A dummy 1-column exp hoists the ~1.3us
    ACT_TABLE_LOAD into the DMA shadow; dummy matmuls ramp the PE p-state.
  - Slot order: smallest piece first (tiny k0 -> earliest exp stream
    start), then descending; a small slot last keeps the tail short.
"""

import numpy as np

import concourse.tile as tile
from concourse import bacc, mybir
from concourse.bass_utils import run_bass_kernel_spmd

B, Q, K, D = 32, 1024, 1024, 128
N_CORES = 8
QH = 512
SCALE = float(1.0 / np.sqrt(D))
N_WARM = 8

LAST_RESULTS = None
_NC_CACHE: dict = {}


# ---------------------------------------------------------------- packing

def _compositions(n, parts):
    """All count-tuples over `parts` (descending) summing to n."""
    out = []

    def rec(i, left, cur):
        if left == 0:
            out.append(tuple(cur + [0] * (len(parts) - len(cur))))
            return
        if i == len(parts):
            return
        for c in range(left // parts[i], -1, -1):
            rec(i + 1, left - c * parts[i], cur + [c])

    rec(0, n, [])
    return out


def _solve_sizes(kbs, sizes, n_cores):
    """Cut batches (block counts kbs) into pieces matching the cell multiset
    {8 x s for s in sizes}. Returns per-batch piece-size lists or None."""
    usizes = sorted(set(sizes), reverse=True)
    cap = tuple(sizes.count(s) * n_cores for s in usizes)
    comps = {kb: _compositions(kb, usizes) for kb in set(kbs)}
    if any(not v for v in comps.values()):
        return None

    from functools import lru_cache

    kbs_t = tuple(kbs)

    @lru_cache(maxsize=None)
    def rec(i, remaining):
        if i == len(kbs_t):
            return ()
        for comp in comps[kbs_t[i]]:
            if any(c > r for c, r in zip(comp, remaining)):
                continue
            sub = rec(i + 1, tuple(r - c for r, c in zip(remaining, comp)))
            if sub is not None:
                return (comp,) + sub
        return None

    res = rec(0, cap)
    if res is None:
        return None
    return [
        [s for s, c in zip(usizes, comp) for _ in range(c)] for comp in res
    ]


def _find_packing(vl, n_cores=N_CORES, max_groups=6):
    """Returns (sizes, per-batch piece lists). sizes sums to the per-core
    block budget. Falls back to the per-slot-max whole-batch scheme."""
    kbs = [max(1, int(np.ceil(v / 128.0))) for v in vl]
    n_total = sum(kbs)
    t0 = int(np.ceil(n_total / n_cores))
    for T in range(t0, t0 + 3):
        cands = []

        def gen(left, maxp, cur):
            if left == 0:
                cands.append(list(cur))
                return
            if len(cur) >= max_groups:
                return
            for p in range(min(maxp, left), 0, -1):
                gen(left - p, p, cur + [p])

        gen(T, K // 128, [])
        cands.sort(key=lambda s: (len(s), -min(s)))
        for sizes in cands:
            sol = _solve_sizes(kbs, sizes, n_cores)
            if sol is not None:
                return sizes, sol
    # fallback: sorted whole-batch slots (baseline scheme)
    order = np.argsort(-np.asarray(vl), kind="stable")
    assign = order.reshape(-1, n_cores)
    sizes = [max(1, int(np.ceil(np.asarray(vl)[assign[j]].max() / 128.0)))
             for j in range(assign.shape[0])]
    sol = [[] for _ in kbs]
    for j in range(assign.shape[0]):
        for b in assign[j]:
            sol[b] = [sizes[j]]
    return sizes, sol


def _order_slots(sizes):
    """Program order: smallest first (fast head), then descending, a small
    one last (short tail). Returns the ordered size list."""
    s = sorted(sizes)
    first = s[0]
    rest = s[1:]
    if len(rest) >= 1:
        last = rest[0]  # next-smallest goes last
        mid = sorted(rest[1:], reverse=True)
        return [first] + mid + [last]
    return [first]


# ---------------------------------------------------------------- device

def _body(tc, qs, ks, vs, oTs, paccs, sizes):
    nc = tc.nc
    f32 = mybir.dt.float32
    f16 = mybir.dt.float16
    AF = mybir.ActivationFunctionType
    G = len(sizes)
    smax = max(sizes)

    with (
        tc.tile_pool(name="q", bufs=3) as q_pool,
        tc.tile_pool(name="k", bufs=3) as k_pool,
        tc.tile_pool(name="v", bufs=3) as v_pool,
        tc.tile_pool(name="p", bufs=2) as p_pool,
        tc.tile_pool(name="pa", bufs=2) as pa_pool,
        tc.tile_pool(name="eo", bufs=2) as e_pool,
        tc.tile_pool(name="const", bufs=1) as c_pool,
        tc.tile_pool(name="spsum", bufs=3, space="PSUM") as s_pool,
        tc.tile_pool(name="opsum", bufs=1, space="PSUM") as o_pool,
    ):
        def load_slot(g):
            s = sizes[g]
            sc = s * 128
            q_sb = q_pool.tile([128, Q], f16, tag="q", name=f"q{g}")
            k_sb = k_pool.tile([128, smax * 128], f16, tag="k", name=f"k{g}")
            v_sb = v_pool.tile([128, smax * 128], f16, tag="v", name=f"v{g}")
            if g == 0:
                # both rings in parallel: k0 (small) + q0h1 on sync, q0h0 on
                # scalar, so the first exp fires as early as possible
                nc.sync.dma_start(out=k_sb[:, 0:sc], in_=ks[g][:])
                nc.scalar.dma_start(out=q_sb[:, 0:QH], in_=qs[g][:, 0:QH])
                nc.sync.dma_start(out=q_sb[:, QH:Q], in_=qs[g][:, QH:Q])
                nc.sync.dma_start(out=v_sb[:, 0:sc], in_=vs[g][:])
            elif g == 1:
                # scalar is still free until the first exp (~4.5us)
                nc.sync.dma_start(out=k_sb[:, 0:sc], in_=ks[g][:])
                nc.scalar.dma_start(out=q_sb[:], in_=qs[g][:])
                nc.sync.dma_start(out=v_sb[:, 0:sc], in_=vs[g][:])
            else:
                nc.sync.dma_start(out=k_sb[:, 0:sc], in_=ks[g][:])
                nc.sync.dma_start(out=q_sb[:], in_=qs[g][:])
                nc.sync.dma_start(out=v_sb[:, 0:sc], in_=vs[g][:])
            return q_sb, k_sb, v_sb

        def s_exp_one(g, blk, q_sb, k_sb, p_all):
            s_ps = s_pool.tile([128, Q], f32, tag="s", name=f"s{g}_{blk}")
            lhsT = k_sb[:, blk * 128 : (blk + 1) * 128]
            for qh in range(Q // QH):
                nc.tensor.matmul(
                    s_ps[:, qh * QH : (qh + 1) * QH],
                    lhsT,
                    q_sb[:, qh * QH : (qh + 1) * QH],
                    start=True,
                    stop=True,
                )
            nc.scalar.activation(
                p_all[:, blk * Q : (blk + 1) * Q], s_ps[:], AF.Exp, scale=SCALE
            )

        def s_exp_head(g, q_sb, k_sb):
            p_all = p_pool.tile([128, smax * Q], f16, tag="p", name=f"p{g}")
            for blk in range(min(2, sizes[g])):
                s_exp_one(g, blk, q_sb, k_sb, p_all)
            return p_all

        def s_exp_tail(g, q_sb, k_sb, p_all, start_blk):
            s = sizes[g]
            if s == 1:
                return None
            pacc = pa_pool.tile([128, Q], f16, tag="pa", name=f"pa{g}")
            nc.vector.tensor_add(pacc[:], p_all[:, 0:Q], p_all[:, Q : 2 * Q])
            for blk in range(2, s):
                if blk >= start_blk:
                    s_exp_one(g, blk, q_sb, k_sb, p_all)
                nc.vector.tensor_add(
                    pacc[:], pacc[:], p_all[:, blk * Q : (blk + 1) * Q]
                )
            return pacc

        def pv_out(g, p_all, v_sb, pacc, nxt=None):
            s = sizes[g]
            last = g == G - 1
            # pacc output: P itself for 1-block pieces (no DVE work)
            nc.sync.dma_start(
                out=paccs[g], in_=(pacc[:] if pacc is not None else p_all[:, 0:Q])
            )
            o_ps = [
                o_pool.tile([128, QH], f32, tag=f"o{qh}", name=f"o{g}_{qh}")
                for qh in range(Q // QH)
            ]
            for blk in range(s):
                for qh in range(Q // QH):
                    nc.tensor.matmul(
                        o_ps[qh][:],
                        v_sb[:, blk * 128 : (blk + 1) * 128],
                        p_all[:, blk * Q + qh * QH : blk * Q + (qh + 1) * QH],
                        start=(blk == 0),
                        stop=(blk == s - 1),
                    )
            # next slot's 3rd S tile + exp ride between PV and the evac so
            # the ScalarE stream stays dense across the slot boundary
            if nxt is not None:
                ng, nq, nk, np_all = nxt
                if sizes[ng] > 2:
                    s_exp_one(ng, 2, nq, nk, np_all)
            o_all = e_pool.tile([128, Q], f16, tag="oall", name=f"oall{g}")
            if last:
                # tail: halves split across DVE and ScalarE (exp stream done),
                # DMAs split across sync and scalar rings
                nc.vector.tensor_copy(o_all[:, 0:QH], o_ps[0][:])
                nc.sync.dma_start(out=oTs[g][:, 0:QH], in_=o_all[:, 0:QH])
                nc.scalar.copy(o_all[:, QH:Q], o_ps[1][:])
                nc.scalar.dma_start(out=oTs[g][:, QH:Q], in_=o_all[:, QH:Q])
            else:
                for qh in range(Q // QH):
                    nc.vector.tensor_copy(
                        o_all[:, qh * QH : (qh + 1) * QH], o_ps[qh][:]
                    )
                nc.sync.dma_start(out=oTs[g], in_=o_all[:])

        # slot-0 loads first so their DMA issues are each ring's first work
        loads = [load_slot(0)]

        # dummy 1-column exp hoists the compiler-inserted ACT_TABLE_LOAD
        # (~1.3us) into the slot-0 DMA shadow
        scratch = c_pool.tile([128, 1], f16, tag="scratch", bufs=1)
        nc.gpsimd.memset(scratch[:], 1.0)
        nc.scalar.activation(scratch[:], scratch[:], AF.Exp, scale=1.0)

        # dummy matmuls ramp the PE p-state while slot-0 loads are in flight
        warm_w = c_pool.tile([128, QH], f16, tag="warmw", bufs=1)
        nc.gpsimd.memset(warm_w[:], 0.0)
        for w in range(N_WARM):
            warm_ps = s_pool.tile([128, QH], f32, tag="s", name=f"warm{w}")
            nc.tensor.matmul(warm_ps[:], warm_w[:, 0:128], warm_w[:],
                             start=True, stop=True)

        p_alls = [s_exp_head(0, loads[0][0], loads[0][1])]
        for g in range(G):
            if g + 1 < G:
                loads.append(load_slot(g + 1))
            q_sb, k_sb, v_sb = loads[g]
            pacc = s_exp_tail(g, q_sb, k_sb, p_alls[g],
                              start_blk=2 if g == 0 else 3)
            nxt = None
            if g + 1 < G:
                lq, lk, lv = loads[g + 1]
                p_alls.append(s_exp_head(g + 1, lq, lk))
                nxt = (g + 1, lq, lk, p_alls[g + 1])
            pv_out(g, p_alls[g], v_sb, pacc, nxt)


def _build(sizes):
    key = tuple(sizes)
    if key in _NC_CACHE:
        return _NC_CACHE[key]
    nc = bacc.Bacc("TRN2", target_bir_lowering=False, debug=False,
                   enable_asserts=False, enable_partition_id=False)
    f16 = mybir.dt.float16
    qs, ks, vs, oTs, paccs = [], [], [], [], []
    for g, s in enumerate(sizes):
        sc = s * 128
        qs.append(nc.dram_tensor(f"q{g}", [D, Q], f16,
                                 kind="ExternalInput").ap())
        ks.append(nc.dram_tensor(f"k{g}", [D, sc], f16,
                                 kind="ExternalInput").ap())
        vs.append(nc.dram_tensor(f"v{g}", [128, sc], f16,
                                 kind="ExternalInput").ap())
        oTs.append(nc.dram_tensor(f"oT{g}", [D, Q], f16,
                                  kind="ExternalOutput").ap())
        paccs.append(nc.dram_tensor(f"pacc{g}", [128, Q], f16,
                                    kind="ExternalOutput").ap())
    with tile.TileContext(nc) as tc:
        _body(tc, qs, ks, vs, oTs, paccs, sizes)
    nc.compile()
    _NC_CACHE[key] = nc
    return nc


# ---------------------------------------------------------------- host

def _prep(queries, keys, values, valid_length):
    """Returns (in_maps, pieces_by_cell, sizes).
    pieces_by_cell[(core, slot)] = (batch, k0_block, n_blocks) or None."""
    vl = np.asarray(valid_length).astype(np.int64).reshape(B)
    sizes_ms, per_batch = _find_packing(vl)
    sizes = _order_slots(sizes_ms)
    G = len(sizes)

    # cut each batch into contiguous pieces (largest piece first at k0=0)
    pieces_by_size: dict[int, list] = {}
    for b in range(B):
        k0 = 0
        for s in sorted(per_batch[b], reverse=True):
            pieces_by_size.setdefault(s, []).append((b, k0, s))
            k0 += s
    # fill cells slot by slot
    cells = {}
    for g in range(G):
        s = sizes[g]
        for c in range(N_CORES):
            lst = pieces_by_size.get(s, [])
            cells[(c, g)] = lst.pop() if lst else None
    assert all(not v for v in pieces_by_size.values()), "unassigned pieces"

    q = np.asarray(queries, dtype=np.float32)
    k = np.asarray(keys, dtype=np.float32)
    v = np.asarray(values, dtype=np.float32)
    qT_all = np.ascontiguousarray(q.transpose(0, 2, 1)).astype(np.float16)
    pos = np.arange(K)

    in_maps = []
    for c in range(N_CORES):
        m = {}
        for g in range(G):
            s = sizes[g]
            sc = s * 128
            piece = cells[(c, g)]
            if piece is None:
                m[f"q{g}"] = np.zeros((D, Q), np.float16)
                m[f"k{g}"] = np.zeros((D, sc), np.float16)
                m[f"v{g}"] = np.zeros((128, sc), np.float16)
                continue
            bi, k0, s_ = piece
            lo, hi = k0 * 128, k0 * 128 + sc
            valid = (pos[lo:hi] < vl[bi])[None, :]
            m[f"q{g}"] = qT_all[bi]
            m[f"k{g}"] = np.where(
                valid, k[bi, lo:hi].T, np.float32(0.0)
            ).astype(np.float16)
            vz = np.where(valid.T, v[bi, lo:hi], np.float32(0.0))
            m[f"v{g}"] = np.ascontiguousarray(
                vz.reshape(s, 128, D).transpose(1, 0, 2).reshape(128, sc)
            ).astype(np.float16)
        in_maps.append(m)
    return in_maps, cells, sizes, vl


def kernel(queries, keys, values, valid_length):
    global LAST_RESULTS
    in_maps, cells, sizes, vl = _prep(queries, keys, values, valid_length)
    nc = _build(sizes)
    res = run_bass_kernel_spmd(nc, in_maps, list(range(N_CORES)))
    LAST_RESULTS = res
    num = np.zeros((B, Q, D), np.float32)
    den = np.zeros((B, Q), np.float32)
    for c in range(N_CORES):
        rc = res.results[c]
        for g in range(len(sizes)):
            piece = cells[(c, g)]
            if piece is None:
                continue
            bi, k0, s = piece
            oT = np.asarray(rc[f"oT{g}"]).astype(np.float32)      # [D, Q]
            pacc = np.asarray(rc[f"pacc{g}"]).astype(np.float32)  # [128, Q]
            n_valid = int(np.clip(vl[bi] - 128 * k0, 0, 128 * s))
            n_masked = 128 * s - n_valid
            num[bi] += oT.T
            den[bi] += pacc.sum(axis=0) - np.float32(n_masked)
    return num / den[:, :, None]
